# revision 23
# baseline (speedup 1.0000x reference)
"""Trainium2 Bass kernel for nn_ChildHAggregation (gnn_message_passing).

Per-sample math (B=32768, HALF=512, DIM=1024):
  x = [hl, hr]; 2-token attention with HyperLinear q/k; layernorm;
  out = hidden(x_norm, xh) + leaf(xw, xh)   (both HyperLinear)

v4 design, pure data-parallel, batch-major [128 x feat] tiles:
  - ALL weight folding is host-side numpy (fp16, pre-rearranged for
    contiguous DMA); no device-side setup compute.
  - hs/hd basis (hs=hl+hr, hd=hl-hr) built in TRANSPOSED space;
    layernorm stats derived from ql/qr/cr2 row accumulations.
  - d0/d1 via the score-difference trick; p00/p11 via two-term softsign
    sigmoid (max err 1.9e-3) using only Square/Sqrt/reciprocal.
  - M-path is attention-free: x@hU_a = hs@WS + p00*(hd@WT) - p11*(hd@WB)
  - xw is transposed in f32 on the PE (2 cycles/row) and downcast in the
    PSUM eviction - no separate xw cast op.
  - 3-stage software pipeline: tile i+1's loads/casts AND transposes are
    emitted before tile i's tail so no engine queue blocks the PE.
  - D-phase PSUM unit order chosen so the 6-bank rotation always reuses
    a bank whose consumer ran early (su_h/su_l/sbc/w1 head of the tail;
    Mb scalar-evicted right after the sigmoid).
All matmul operands fp16 (same PE speed as bf16, 8x finer mantissa).
"""

from contextlib import ExitStack

import numpy as np

import concourse.bacc as bacc
import concourse.bass as bass
import concourse.mybir as mybir
import concourse.tile as tile
from concourse.bass_utils import run_bass_kernel_spmd
from concourse.masks import make_identity

N_CORES = 8
HALF = 512
DIM = 1024
P = 128
IS = 1.0 / float(np.sqrt(np.float32(HALF)))

# two-term softsign sigmoid constants (max |err| 1.9e-3 over |z|<=14)
SIG_A1 = 2.057838
SIG_C1 = 8.347378
SIG_A2 = 0.5 - SIG_A1
SIG_C2 = 11.527823
SIG_K1 = SIG_A1 * IS / float(np.sqrt(SIG_C1))
SIG_K2 = SIG_A2 * IS / float(np.sqrt(SIG_C2))

f32 = mybir.dt.float32
fp16 = mybir.dt.float16

ALU = mybir.AluOpType
ACTF = mybir.ActivationFunctionType

W4 = ["qWu3", "qWbF", "kWu3", "kU3", "qUT3", "hWu3", "lWu3", "WC3",
      "WS", "WT", "WB"]
BCN = ["b_qWu", "b_kWu", "b_qb", "b_cs512", "b_hWu", "b_lWu", "b_cb"]


def _r4(w):
    return np.ascontiguousarray(
        w.reshape(4, P, HALF).transpose(1, 0, 2).astype(np.float16))


def _r8(w):
    return np.ascontiguousarray(
        w.reshape(8, P, HALF).transpose(1, 0, 2).astype(np.float16))


def _bc(row):
    return np.ascontiguousarray(
        np.broadcast_to(row.astype(np.float16)[None, :], (P, HALF)))


def host_prep(inputs):
    """Fold weights/biases in f32 numpy; emit fp16 device buffers."""
    g = {k: np.asarray(v, dtype=np.float32) for k, v in inputs.items()}
    out = {}
    out["qUT3"] = _r4(np.ascontiguousarray(g["qU_w"].T))
    out["kU3"] = _r4(g["kU_w"])
    out["qWu3"] = _r4(g["qWu_w"])
    out["kWu3"] = _r4(g["kWu_w"])
    out["qWbF"] = _r4(g["qWb_w"] + g["qWu_w"] * g["qU_b"][None, :])
    hU_a = g["hU_w"] * g["alpha"][:, None]
    out["WS"] = _r4(hU_a[:HALF] + hU_a[HALF:])
    out["WT"] = _r4(hU_a[:HALF])
    out["WB"] = _r4(hU_a[HALF:])
    out["hWu3"] = _r4(g["hWu_w"])
    out["lWu3"] = _r4(g["lWu_w"])
    bh = g["beta"] @ g["hU_w"] + g["hU_b"]
    out["WC3"] = _r4(g["hWb_w"] + g["hWu_w"] * bh[None, :]
                     + g["lWb_w"] + g["lWu_w"] * g["lU_b"][None, :])
    out["lU3"] = _r8(g["lU_w"])
    out["b_qWu"] = _bc(g["qWu_b"])
    out["b_kWu"] = _bc(g["kWu_b"])
    out["b_qb"] = _bc(g["qWb_b"] + g["qU_b"] * g["qWu_b"])
    out["b_cs512"] = _bc((g["alpha"] @ g["hU_w"]) / 512.0)
    out["b_hWu"] = _bc(g["hWu_b"])
    out["b_lWu"] = _bc(g["lWu_b"])
    out["b_cb"] = _bc(g["hWb_b"] + bh * g["hWu_b"]
                      + g["lWb_b"] + g["lU_b"] * g["lWu_b"])
    return out


def build_nc(b_loc):
    n_tiles = b_loc // P
    assert n_tiles * P == b_loc

    nc = bacc.Bacc("TRN2", target_bir_lowering=False, debug=False,
                   num_devices=1)

    d = {}
    d["hl"] = nc.dram_tensor("hl", [b_loc, HALF], f32, kind="ExternalInput").ap()
    d["hr"] = nc.dram_tensor("hr", [b_loc, HALF], f32, kind="ExternalInput").ap()
    d["xw"] = nc.dram_tensor("xw", [b_loc, DIM], f32, kind="ExternalInput").ap()
    d["xh"] = nc.dram_tensor("xh", [b_loc, HALF], f32, kind="ExternalInput").ap()
    for w in W4:
        d[w] = nc.dram_tensor(w, [P, 4, HALF], fp16, kind="ExternalInput").ap()
    d["lU3"] = nc.dram_tensor("lU3", [P, 8, HALF], fp16,
                              kind="ExternalInput").ap()
    for w in BCN:
        d[w] = nc.dram_tensor(w, [P, HALF], fp16, kind="ExternalInput").ap()
    out_d = nc.dram_tensor("out", [b_loc, HALF], f32, kind="ExternalOutput").ap()

    with tile.TileContext(nc) as tc, ExitStack() as ctx:
        wts = ctx.enter_context(tc.tile_pool(name="wts", bufs=1))
        ident = wts.tile([P, P], fp16)
        make_identity(nc, ident)
        ident32 = wts.tile([P, P], f32)
        make_identity(nc, ident32)
        cb2 = wts.tile([P, 2], f32)
        nc.vector.memset(cb2[:, 0:1], 0.5)
        nc.vector.memset(cb2[:, 1:2], -0.5)
        wsb = {}
        bc = {}

        def load_weights():
            for w in W4:
                wsb[w] = wts.tile([P, 4, HALF], fp16, name=f"w_{w}")
                nc.sync.dma_start(wsb[w], d[w])
            wsb["lU3"] = wts.tile([P, 8, HALF], fp16, name="w_lU3")
            nc.sync.dma_start(wsb["lU3"], d["lU3"])
            for w in BCN:
                bc[w] = wts.tile([P, HALF], fp16, name=f"bc_{w}")
                nc.sync.dma_start(bc[w], d[w])

        inp = ctx.enter_context(tc.tile_pool(name="inp", bufs=4))
        b16 = ctx.enter_context(tc.tile_pool(name="b16", bufs=4))
        tsp = ctx.enter_context(tc.tile_pool(name="tsp", bufs=3))
        scr = ctx.enter_context(tc.tile_pool(name="scr", bufs=3))
        att = ctx.enter_context(tc.tile_pool(name="att", bufs=2))
        tinyp = ctx.enter_context(tc.tile_pool(name="tinyp", bufs=4))
        phd = ctx.enter_context(tc.tile_pool(name="phd", bufs=2))
        outp = ctx.enter_context(tc.tile_pool(name="outp", bufs=2))
        tp_ps = ctx.enter_context(tc.tile_pool(name="tp_ps", bufs=2,
                                               space="PSUM"))
        mm_ps = ctx.enter_context(tc.tile_pool(name="mm_ps", bufs=6,
                                               space="PSUM"))

        st0 = {}
        stT = {}

        def stage0(i):
            """Loads + hl/hr/xh downcasts (+ sl/sr accums)."""
            rs = bass.ts(i, P)
            hl_t = inp.tile([P, HALF], f32, tag="hl", name=f"hl_{i}")
            nc.sync.dma_start(hl_t, d["hl"][rs, :])
            hr_t = inp.tile([P, HALF], f32, tag="hr", name=f"hr_{i}")
            nc.sync.dma_start(hr_t, d["hr"][rs, :])
            xh_t = inp.tile([P, HALF], f32, tag="xh", name=f"xh_{i}")
            nc.sync.dma_start(xh_t, d["xh"][rs, :])
            xw_t = inp.tile([P, DIM], f32, tag="xw", name=f"xw_{i}")
            nc.sync.dma_start(xw_t, d["xw"][rs, :])

            sS = tinyp.tile([P, 2], f32, tag="sS", name=f"sS_{i}")
            hl_b = b16.tile([P, HALF], fp16, tag="hlb", name=f"hlb_{i}")
            nc.scalar.activation(hl_b, hl_t, ACTF.Copy, accum_out=sS[:, 0:1])
            hr_b = b16.tile([P, HALF], fp16, tag="hrb", name=f"hrb_{i}")
            nc.scalar.activation(hr_b, hr_t, ACTF.Copy, accum_out=sS[:, 1:2])
            xh_b = b16.tile([P, HALF], fp16, tag="xhb", name=f"xhb_{i}")
            nc.scalar.copy(xh_b, xh_t)
            st0[i] = (hl_t, hr_t, xh_t, xw_t, hl_b, hr_b, xh_b, sS)

        def stageT(i):
            """PE transposes + scalar evicts + hs/hd basis (vector)."""
            (hl_t, hr_t, xh_t, xw_t, hl_b, hr_b, xh_b, sS) = st0[i]

            def tp16(src, tg):
                sb = tsp.tile([P, 4 * P], fp16, tag=tg, name=f"T_{tg}_{i}")
                ps = tp_ps.tile([P, 4 * P], fp16, tag="tp",
                                name=f"tps_{tg}_{i}")
                for c in range(4):
                    nc.tensor.transpose(ps[:, c * P:(c + 1) * P],
                                        src[:, c * P:(c + 1) * P], ident)
                nc.scalar.copy(sb, ps)
                return sb

            hlT = tp16(hl_b, "ThL")
            hrT = tp16(hr_b, "ThR")
            hsT = tsp.tile([P, 4 * P], fp16, tag="ThS", name=f"T_ThS_{i}")
            nc.vector.tensor_add(hsT, hlT, hrT)
            hdT = tsp.tile([P, 4 * P], fp16, tag="ThD", name=f"T_ThD_{i}")
            nc.vector.tensor_sub(hdT, hlT, hrT)
            # xh: chunk-wise DMA XBAR transposes (16-bit, contiguous dst)
            xhT = []
            for c in range(4):
                t = tsp.tile([P, P], fp16, tag=f"TxH{c}", name=f"TxH{c}_{i}")
                nc.sync.dma_start_transpose(t, xh_b[:, c * P:(c + 1) * P])
                xhT.append(t)
            # xw: f32 transpose, downcast in the eviction.  Half-width
            # f32 PSUM groups (1KB) share the fp16 groups' pool slot.
            xwT = tsp.tile([P, 8 * P], fp16, tag="TxW", name=f"T_TxW_{i}")
            for g in range(4):
                ps = tp_ps.tile([P, 2 * P], f32, tag="tp",
                                name=f"tpw_{g}_{i}")
                for c in range(2):
                    nc.tensor.transpose(
                        ps[:, c * P:(c + 1) * P],
                        xw_t[:, (2 * g + c) * P:(2 * g + c + 1) * P], ident32)
                nc.scalar.copy(xwT[:, g * 2 * P:(g + 1) * 2 * P], ps)
            stT[i] = (hsT, hdT, xhT, xwT)

        def unit(tag, i):
            return mm_ps.tile([P, HALF], f32, tag="mm", name=f"ps_{tag}_{i}")

        def stage1m(i):
            """A+D matmuls and A-phase elementwise."""
            (hl_t, hr_t, xh_t, xw_t, hl_b, hr_b, xh_b, sS) = st0[i]
            (hsT, hdT, xhT, xwT) = stT[i]

            qS = tinyp.tile([P, 8], f32, tag="qS", name=f"qS_{i}")
            # ql/qr on scalar (Square + accum), cr2 on vector
            sg1 = scr.tile([P, HALF], fp16, tag="scr", name=f"scr_ql_{i}")
            nc.scalar.activation(sg1, hl_t, ACTF.Square,
                                 accum_out=qS[:, 0:1])
            sg2 = scr.tile([P, HALF], fp16, tag="scr", name=f"scr_qr_{i}")
            nc.scalar.activation(sg2, hr_t, ACTF.Square,
                                 accum_out=qS[:, 1:2])
            sg3 = scr.tile([P, HALF], fp16, tag="scr", name=f"scr_cr_{i}")
            nc.vector.scalar_tensor_tensor(sg3, hl_t, 0.0, hr_t, ALU.bypass,
                                           ALU.mult, accum_out=qS[:, 2:3])

            SUq, SBq, TU = unit("SUq", i), unit("SBq", i), unit("TU", i)
            for c in range(4):
                lhs = xhT[c]
                st, sp_ = (c == 0), (c == 3)
                nc.tensor.matmul(SUq, lhs, wsb["qWu3"][:, c, :], start=st, stop=sp_)
                nc.tensor.matmul(SBq, lhs, wsb["qWbF"][:, c, :], start=st, stop=sp_)
                nc.tensor.matmul(TU, lhs, wsb["kWu3"][:, c, :], start=st, stop=sp_)
            CD = unit("CD", i)
            for c in range(4):
                nc.tensor.matmul(CD, hdT[:, bass.ts(c, P)],
                                 wsb["kU3"][:, c, :], start=(c == 0),
                                 stop=(c == 3))

            su = att.tile([P, HALF], fp16, tag="su", name=f"su_{i}")
            nc.vector.tensor_add(su, SUq, bc["b_qWu"])
            sbq = att.tile([P, HALF], fp16, tag="sbq", name=f"sbq_{i}")
            nc.vector.tensor_add(sbq, SBq, bc["b_qb"])
            tu = att.tile([P, HALF], fp16, tag="tu", name=f"tu_{i}")
            nc.vector.tensor_add(tu, TU, bc["b_kWu"])
            dk = att.tile([P, HALF], fp16, tag="dk", name=f"dk_{i}")
            nc.vector.tensor_mul(dk, CD, tu)
            u = att.tile([P, HALF], fp16, tag="u", name=f"u_{i}")
            nc.gpsimd.tensor_mul(u, su, dk)
            # uT for the Gv unit, via DMA XBAR
            uT = []
            for c in range(4):
                t = tsp.tile([P, P], fp16, tag=f"TuT{c}", name=f"TuT{c}_{i}",
                             bufs=2)
                nc.sync.dma_start_transpose(t, u[:, c * P:(c + 1) * P])
                uT.append(t)

            # c-dot; hl/hr dots come after the Gv unit below
            sdot0 = scr.tile([P, HALF], fp16, tag="scr_b",
                             name=f"scr_dot0_{i}")
            nc.vector.scalar_tensor_tensor(
                sdot0, sbq, 0.0, dk, ALU.bypass, ALU.mult,
                accum_out=qS[:, 3:4])

            # D-phase (attention-independent). Unit order tuned for the
            # 6-slot PSUM rotation: each alloc reuses a bank whose
            # consumer runs early.
            HSU, LSU = unit("HSU", i), unit("LSU", i)
            SBC = unit("SBC", i)
            for c in range(4):
                lhs = xhT[c]
                st, sp_ = (c == 0), (c == 3)
                nc.tensor.matmul(HSU, lhs, wsb["hWu3"][:, c, :], start=st, stop=sp_)
                nc.tensor.matmul(LSU, lhs, wsb["lWu3"][:, c, :], start=st, stop=sp_)
                nc.tensor.matmul(SBC, lhs, wsb["WC3"][:, c, :], start=st, stop=sp_)
            # drain the D hyper units early (frees banks for the rotation)
            su_h = phd.tile([P, HALF], fp16, tag="su_h", name=f"su_h_{i}")
            nc.vector.tensor_add(su_h, HSU, bc["b_hWu"])
            su_l = phd.tile([P, HALF], fp16, tag="su_l", name=f"su_l_{i}")
            nc.vector.tensor_add(su_l, LSU, bc["b_lWu"])
            sbc = phd.tile([P, HALF], fp16, tag="sbc", name=f"sbc_{i}")
            nc.vector.tensor_add(sbc, SBC, bc["b_cb"])

            LUp = unit("LU", i)
            for c in range(8):
                nc.tensor.matmul(LUp, xwT[:, bass.ts(c, P)],
                                 wsb["lU3"][:, c, :], start=(c == 0),
                                 stop=(c == 7))
            Mb = unit("Mb", i)
            for c in range(4):
                nc.tensor.matmul(Mb, hsT[:, bass.ts(c, P)],
                                 wsb["WS"][:, c, :], start=(c == 0),
                                 stop=(c == 3))
            D1 = unit("D1", i)
            for c in range(4):
                nc.tensor.matmul(D1, hdT[:, bass.ts(c, P)],
                                 wsb["WT"][:, c, :], start=(c == 0),
                                 stop=(c == 3))
            D2 = unit("D2", i)
            for c in range(4):
                nc.tensor.matmul(D2, hdT[:, bass.ts(c, P)],
                                 wsb["WB"][:, c, :], start=(c == 0),
                                 stop=(c == 3))
            w1 = phd.tile([P, HALF], fp16, tag="w1", name=f"w1_{i}")
            nc.vector.tensor_mul(w1, LUp, su_l)

            # Gv = u @ qU.T; then d0 = c + hl.Gv, d1 = c + hr.Gv
            Gv = unit("Gv", i)
            for c in range(4):
                nc.tensor.matmul(Gv, uT[c], wsb["qUT3"][:, c, :],
                                 start=(c == 0), stop=(c == 3))
            for j, hh in enumerate([hl_t, hr_t]):
                sdot = scr.tile([P, HALF], fp16, tag="scr_b",
                                name=f"scr_dot{j + 1}_{i}")
                nc.vector.scalar_tensor_tensor(
                    sdot, Gv, 0.0, hh, ALU.bypass, ALU.mult,
                    accum_out=qS[:, 4 + j:5 + j])
            return (qS, sS, su_h, su_l, sbc, w1, Mb, D1, D2)

        def stage1b(i, h):
            (qS, sS, su_h, su_l, sbc, w1, Mb, D1, D2) = h
            ql, qr, cr2 = qS[:, 0:1], qS[:, 1:2], qS[:, 2:3]
            cdt = qS[:, 3:4]
            sl, sr = sS[:, 0:1], sS[:, 1:2]

            # Mb eviction on scalar (frees its bank early; also needed
            # because DVE reads at most one PSUM operand per op)
            mb_sb = phd.tile([P, HALF], f32, tag="mb_sb", name=f"mb_{i}")
            nc.scalar.copy(mb_sb, Mb)

            # d = [d0, d1] = [c + hl.Gv, c + hr.Gv]; two-term softsign
            # sigmoid -> ab = [p00, -p11]
            dd = tinyp.tile([P, 2], f32, tag="dd", name=f"dd_{i}")
            nc.vector.scalar_tensor_tensor(dd, qS[:, 4:6], 0.0,
                                           cdt.broadcast_to([P, 2]),
                                           ALU.bypass, ALU.add)
            z2 = tinyp.tile([P, 2], f32, tag="z2", name=f"z2_{i}")
            nc.scalar.activation(z2, dd, ACTF.Square, scale=IS)
            sq1 = tinyp.tile([P, 2], f32, tag="sq1", name=f"sq1_{i}")
            nc.scalar.activation(sq1, z2, ACTF.Sqrt, scale=1.0 / SIG_C1,
                                 bias=1.0)
            sq2 = tinyp.tile([P, 2], f32, tag="sq2", name=f"sq2_{i}")
            nc.scalar.activation(sq2, z2, ACTF.Sqrt, scale=1.0 / SIG_C2,
                                 bias=1.0)
            rr = tinyp.tile([P, 4], f32, tag="rr", name=f"rr_{i}")
            nc.vector.reciprocal(rr[:, 0:2], sq1)
            nc.vector.reciprocal(rr[:, 2:4], sq2)
            mm_ = tinyp.tile([P, 2], f32, tag="mm2", name=f"mm2_{i}")
            nc.vector.scalar_tensor_tensor(mm_, rr[:, 0:2], SIG_K1 / SIG_K2,
                                           rr[:, 2:4], ALU.mult, ALU.add)
            psh = tinyp.tile([P, 2], f32, tag="psh", name=f"psh_{i}")
            nc.vector.scalar_tensor_tensor(psh, dd, SIG_K2, mm_,
                                           ALU.mult, ALU.mult)
            ab = tinyp.tile([P, 2], f32, tag="ab", name=f"ab_{i}")
            nc.vector.tensor_add(ab, psh, cb2)
            al, be = ab[:, 0:1], ab[:, 1:2]

            # D1/D2 drained by scalar as soon as al/be exist
            th0 = phd.tile([P, HALF], fp16, tag="th0", name=f"th0_{i}")
            nc.scalar.activation(th0, D1, ACTF.Copy, scale=al)
            th1 = phd.tile([P, HALF], fp16, tag="th1", name=f"th1_{i}")
            nc.scalar.activation(th1, D2, ACTF.Copy, scale=be)
            hv = phd.tile([P, HALF], fp16, tag="hv", name=f"hv_{i}")
            nc.gpsimd.tensor_add(hv, th0, th1)

            # layernorm stats
            gt = tinyp.tile([P, 8], f32, tag="gt", name=f"gt_{i}")
            g_, gh, dl, base = gt[:, 0:1], gt[:, 1:2], gt[:, 2:3], gt[:, 3:4]
            cA, cB, cC = gt[:, 4:5], gt[:, 5:6], gt[:, 6:7]
            nc.vector.tensor_add(g_, al, be)
            nc.vector.tensor_scalar(gh, g_, 0.5, None, ALU.mult)
            sqab = tinyp.tile([P, 2], f32, tag="sqab", name=f"sqab_{i}")
            nc.vector.tensor_mul(sqab, ab, ab)
            nc.vector.tensor_add(dl, sqab[:, 0:1], sqab[:, 1:2])
            nc.vector.tensor_scalar(base, dl, 0.5, 1.0, ALU.mult, ALU.add)
            nc.vector.tensor_add(cA, base, g_)
            nc.vector.scalar_tensor_tensor(cB, g_, -1.0, base, ALU.mult,
                                           ALU.add)
            nc.vector.tensor_scalar(cC, dl, -1.0, 2.0, ALU.mult, ALU.add)
            acc = tinyp.tile([P, 8], f32, tag="acc", name=f"acc_{i}")
            z0, z1, ssqh = acc[:, 0:1], acc[:, 1:2], acc[:, 2:3]
            sh2, sd2, sumxh = acc[:, 3:4], acc[:, 4:5], acc[:, 5:6]
            m2, varh = acc[:, 6:7], acc[:, 7:8]
            nc.vector.tensor_scalar_mul(z0, cr2, cC)
            nc.vector.scalar_tensor_tensor(z1, ql, cA, z0, ALU.mult, ALU.add)
            nc.vector.scalar_tensor_tensor(ssqh, qr, cB, z1, ALU.mult,
                                           ALU.add)
            nc.vector.tensor_add(sh2, sl, sr)
            nc.vector.tensor_sub(sd2, sl, sr)
            nc.vector.scalar_tensor_tensor(sumxh, sd2, gh, sh2, ALU.mult,
                                           ALU.add)
            nc.vector.tensor_mul(m2, sumxh, sumxh)
            nc.vector.scalar_tensor_tensor(varh, m2, -1.0 / 512.0, ssqh,
                                           ALU.mult, ALU.add)
            so = tinyp.tile([P, 2], f32, tag="so", name=f"so_{i}")
            sqstd, rinv = so[:, 0:1], so[:, 1:2]
            nc.scalar.activation(sqstd, varh, ACTF.Sqrt,
                                 scale=2.0 / (DIM - 1))
            nc.vector.reciprocal(rinv, sqstd)

            # t5 = (Mb + hv) - cs*mean ; u1 = rinv * t5
            t5a = phd.tile([P, HALF], f32, tag="t5a", name=f"t5a_{i}")
            nc.vector.scalar_tensor_tensor(t5a, bc["b_cs512"], sumxh, mb_sb,
                                           ALU.mult, ALU.subtract)
            t5 = phd.tile([P, HALF], fp16, tag="t5", name=f"t5_{i}")
            nc.vector.tensor_sub(t5, hv, t5a)
            u1 = phd.tile([P, HALF], fp16, tag="u1", name=f"u1_{i}")
            nc.scalar.activation(u1, t5, ACTF.Copy, scale=rinv)

            v1 = phd.tile([P, HALF], fp16, tag="v1", name=f"v1_{i}")
            nc.gpsimd.tensor_mul(v1, u1, su_h)
            s2 = phd.tile([P, HALF], fp16, tag="s2", name=f"s2_{i}")
            nc.gpsimd.tensor_add(s2, v1, sbc)
            out_t = outp.tile([P, HALF], f32, tag="out_t", name=f"out_{i}")
            nc.gpsimd.tensor_add(out_t, s2, w1)
            nc.sync.dma_start(out_d[bass.ts(i, P), :], out_t)

        stage0(0)
        stage0(1)
        load_weights()
        stageT(0)
        stageT(1)
        for i in range(n_tiles):
            h = stage1m(i)
            if i + 2 < n_tiles:
                stage0(i + 2)
                stageT(i + 2)
            stage1b(i, h)

    nc.compile()
    return nc


_NC_CACHE = {}


def _get_nc(b_loc, mm_dt=None):
    if b_loc not in _NC_CACHE:
        _NC_CACHE[b_loc] = build_nc(b_loc)
    return _NC_CACHE[b_loc]


def make_in_maps(inputs):
    b = inputs["hl"].shape[0]
    b_loc = b // N_CORES
    prep = host_prep(inputs)
    in_maps = []
    for i in range(N_CORES):
        m = {}
        for k in ("hl", "hr", "xw", "xh"):
            v = np.ascontiguousarray(np.asarray(inputs[k], dtype=np.float32))
            m[k] = v[i * b_loc:(i + 1) * b_loc]
        m.update(prep)
        in_maps.append(m)
    return in_maps


def kernel(**inputs):
    b = inputs["hl"].shape[0]
    nc = _get_nc(b // N_CORES)
    in_maps = make_in_maps(inputs)
    res = run_bass_kernel_spmd(nc, in_maps, core_ids=list(range(N_CORES)))
    return np.concatenate([r["out"] for r in res.results], axis=0)


# revision 26
# speedup vs baseline: 1.5295x; 1.5295x over previous
"""Trainium2 Bass kernel for nn_ChildHAggregation (gnn_message_passing).

Per-sample math (B=32768, HALF=512, DIM=1024):
  x = [hl, hr]; 2-token attention with HyperLinear q/k; layernorm;
  out = hidden(x_norm, xh) + leaf(xw, xh)   (both HyperLinear)

v4 design, pure data-parallel, batch-major [128 x feat] tiles:
  - ALL weight folding is host-side numpy (fp16, pre-rearranged for
    contiguous DMA); no device-side setup compute.
  - hs/hd basis (hs=hl+hr, hd=hl-hr) built in TRANSPOSED space;
    layernorm stats derived from ql/qr/cr2 row accumulations.
  - d0/d1 via the score-difference trick; p00/p11 via two-term softsign
    sigmoid (max err 1.9e-3) using only Square/Sqrt/reciprocal.
  - M-path is attention-free: x@hU_a = hs@WS + p00*(hd@WT) - p11*(hd@WB)
  - xw is transposed in f32 on the PE (2 cycles/row) and downcast in the
    PSUM eviction - no separate xw cast op.
  - 3-stage software pipeline: tile i+1's loads/casts AND transposes are
    emitted before tile i's tail so no engine queue blocks the PE.
  - D-phase PSUM unit order chosen so the 6-bank rotation always reuses
    a bank whose consumer ran early (su_h/su_l/sbc/w1 head of the tail;
    Mb scalar-evicted right after the sigmoid).
All matmul operands fp16 (same PE speed as bf16, 8x finer mantissa).
"""

from contextlib import ExitStack

import numpy as np

import concourse.bacc as bacc
import concourse.bass as bass
import concourse.mybir as mybir
import concourse.tile as tile
from concourse.bass_utils import run_bass_kernel_spmd
from concourse.masks import make_identity

N_CORES = 8
HALF = 512
DIM = 1024
P = 128
IS = 1.0 / float(np.sqrt(np.float32(HALF)))

# two-term softsign sigmoid constants (max |err| 1.9e-3 over |z|<=14)
SIG_A1 = 2.057838
SIG_C1 = 8.347378
SIG_A2 = 0.5 - SIG_A1
SIG_C2 = 11.527823
SIG_K1 = SIG_A1 * IS / float(np.sqrt(SIG_C1))
SIG_K2 = SIG_A2 * IS / float(np.sqrt(SIG_C2))

f32 = mybir.dt.float32
fp16 = mybir.dt.float16

ALU = mybir.AluOpType
ACTF = mybir.ActivationFunctionType

W4 = ["qWu3", "qWbF", "kWu3", "kU3", "qUT3", "hWu3", "lWu3", "WC3",
      "WS", "WT", "WB"]
BCN = ["b_qWu", "b_kWu", "b_qb", "b_cs512", "b_hWu", "b_lWu", "b_cb"]


def _r4(w):
    return np.ascontiguousarray(
        w.reshape(4, P, HALF).transpose(1, 0, 2).astype(np.float16))


def _r8(w):
    return np.ascontiguousarray(
        w.reshape(8, P, HALF).transpose(1, 0, 2).astype(np.float16))


def _bc(row):
    return np.ascontiguousarray(
        np.broadcast_to(row.astype(np.float16)[None, :], (P, HALF)))


def host_prep(inputs):
    """Fold weights/biases in f32 numpy; emit fp16 device buffers."""
    g = {k: np.asarray(v, dtype=np.float32) for k, v in inputs.items()}
    out = {}
    out["qUT3"] = _r4(np.ascontiguousarray(g["qU_w"].T))
    out["kU3"] = _r4(g["kU_w"])
    out["qWu3"] = _r4(g["qWu_w"])
    out["kWu3"] = _r4(g["kWu_w"])
    out["qWbF"] = _r4(g["qWb_w"] + g["qWu_w"] * g["qU_b"][None, :])
    hU_a = g["hU_w"] * g["alpha"][:, None]
    out["WS"] = _r4(hU_a[:HALF] + hU_a[HALF:])
    out["WT"] = _r4(hU_a[:HALF])
    out["WB"] = _r4(hU_a[HALF:])
    out["hWu3"] = _r4(g["hWu_w"])
    out["lWu3"] = _r4(g["lWu_w"])
    bh = g["beta"] @ g["hU_w"] + g["hU_b"]
    out["WC3"] = _r4(g["hWb_w"] + g["hWu_w"] * bh[None, :]
                     + g["lWb_w"] + g["lWu_w"] * g["lU_b"][None, :])
    out["lU3"] = _r8(g["lU_w"])
    out["b_qWu"] = _bc(g["qWu_b"])
    out["b_kWu"] = _bc(g["kWu_b"])
    out["b_qb"] = _bc(g["qWb_b"] + g["qU_b"] * g["qWu_b"])
    out["b_cs512"] = _bc((g["alpha"] @ g["hU_w"]) / 512.0)
    out["b_hWu"] = _bc(g["hWu_b"])
    out["b_lWu"] = _bc(g["lWu_b"])
    out["b_cb"] = _bc(g["hWb_b"] + bh * g["hWu_b"]
                      + g["lWb_b"] + g["lU_b"] * g["lWu_b"])
    return out


def build_nc(b_loc):
    n_tiles = b_loc // P
    assert n_tiles * P == b_loc

    nc = bacc.Bacc("TRN2", target_bir_lowering=False, debug=False,
                   num_devices=1)

    d = {}
    d["hl"] = nc.dram_tensor("hl", [b_loc, HALF], f32, kind="ExternalInput").ap()
    d["hr"] = nc.dram_tensor("hr", [b_loc, HALF], f32, kind="ExternalInput").ap()
    d["xw"] = nc.dram_tensor("xw", [b_loc, DIM], f32, kind="ExternalInput").ap()
    d["xh"] = nc.dram_tensor("xh", [b_loc, HALF], f32, kind="ExternalInput").ap()
    for w in W4:
        d[w] = nc.dram_tensor(w, [P, 4, HALF], fp16, kind="ExternalInput").ap()
    d["lU3"] = nc.dram_tensor("lU3", [P, 8, HALF], fp16,
                              kind="ExternalInput").ap()
    for w in BCN:
        d[w] = nc.dram_tensor(w, [P, HALF], fp16, kind="ExternalInput").ap()
    out_d = nc.dram_tensor("out", [b_loc, HALF], f32, kind="ExternalOutput").ap()

    with tile.TileContext(nc) as tc, ExitStack() as ctx:
        wts = ctx.enter_context(tc.tile_pool(name="wts", bufs=1))
        ident = wts.tile([P, P], fp16)
        make_identity(nc, ident)
        ident32 = wts.tile([P, P], f32)
        make_identity(nc, ident32)
        cb2 = wts.tile([P, 2], f32)
        nc.vector.memset(cb2[:, 0:1], 0.5)
        nc.vector.memset(cb2[:, 1:2], -0.5)
        wsb = {}
        bc = {}

        def load_weights():
            for w in W4:
                wsb[w] = wts.tile([P, 4, HALF], fp16, name=f"w_{w}")
                nc.sync.dma_start(wsb[w], d[w])
            wsb["lU3"] = wts.tile([P, 8, HALF], fp16, name="w_lU3")
            nc.sync.dma_start(wsb["lU3"], d["lU3"])
            for w in BCN:
                bc[w] = wts.tile([P, HALF], fp16, name=f"bc_{w}")
                nc.sync.dma_start(bc[w], d[w])

        inp = ctx.enter_context(tc.tile_pool(name="inp", bufs=4))
        b16 = ctx.enter_context(tc.tile_pool(name="b16", bufs=4))
        tsp = ctx.enter_context(tc.tile_pool(name="tsp", bufs=3))
        scr = ctx.enter_context(tc.tile_pool(name="scr", bufs=3))
        att = ctx.enter_context(tc.tile_pool(name="att", bufs=2))
        tinyp = ctx.enter_context(tc.tile_pool(name="tinyp", bufs=4))
        phd = ctx.enter_context(tc.tile_pool(name="phd", bufs=2))
        outp = ctx.enter_context(tc.tile_pool(name="outp", bufs=2))
        tp_ps = ctx.enter_context(tc.tile_pool(name="tp_ps", bufs=2,
                                               space="PSUM"))
        mm_ps = ctx.enter_context(tc.tile_pool(name="mm_ps", bufs=6,
                                               space="PSUM"))

        st0 = {}
        stT = {}

        def stage0(i):
            """Loads + hl/hr/xh downcasts (+ sl/sr accums)."""
            rs = bass.ts(i, P)
            hl_t = inp.tile([P, HALF], f32, tag="hl", name=f"hl_{i}")
            nc.sync.dma_start(hl_t, d["hl"][rs, :])
            hr_t = inp.tile([P, HALF], f32, tag="hr", name=f"hr_{i}")
            nc.sync.dma_start(hr_t, d["hr"][rs, :])
            xh_t = inp.tile([P, HALF], f32, tag="xh", name=f"xh_{i}")
            nc.sync.dma_start(xh_t, d["xh"][rs, :])
            xw_t = inp.tile([P, DIM], f32, tag="xw", name=f"xw_{i}")
            nc.sync.dma_start(xw_t, d["xw"][rs, :])

            sS = tinyp.tile([P, 2], f32, tag="sS", name=f"sS_{i}")
            hl_b = b16.tile([P, HALF], fp16, tag="hlb", name=f"hlb_{i}")
            nc.scalar.activation(hl_b, hl_t, ACTF.Copy, accum_out=sS[:, 0:1])
            hr_b = b16.tile([P, HALF], fp16, tag="hrb", name=f"hrb_{i}")
            nc.scalar.activation(hr_b, hr_t, ACTF.Copy, accum_out=sS[:, 1:2])
            xh_b = b16.tile([P, HALF], fp16, tag="xhb", name=f"xhb_{i}")
            nc.scalar.copy(xh_b, xh_t)
            st0[i] = (hl_t, hr_t, xh_t, xw_t, hl_b, hr_b, xh_b, sS)

        def stageT(i):
            """PE transposes + scalar evicts + hs/hd basis (vector)."""
            (hl_t, hr_t, xh_t, xw_t, hl_b, hr_b, xh_b, sS) = st0[i]

            def tp16(src, tg):
                sb = tsp.tile([P, 4 * P], fp16, tag=tg, name=f"T_{tg}_{i}")
                ps = tp_ps.tile([P, 4 * P], fp16, tag="tp",
                                name=f"tps_{tg}_{i}")
                for c in range(4):
                    nc.tensor.transpose(ps[:, c * P:(c + 1) * P],
                                        src[:, c * P:(c + 1) * P], ident)
                nc.scalar.copy(sb, ps)
                return sb

            hlT = tp16(hl_b, "ThL")
            hrT = tp16(hr_b, "ThR")
            hsT = tsp.tile([P, 4 * P], fp16, tag="ThS", name=f"T_ThS_{i}")
            nc.vector.tensor_add(hsT, hlT, hrT)
            hdT = tsp.tile([P, 4 * P], fp16, tag="ThD", name=f"T_ThD_{i}")
            nc.vector.tensor_sub(hdT, hlT, hrT)
            xhT = tp16(xh_b, "TxH")
            # xw: f32 transpose, downcast in the eviction.  Half-width
            # f32 PSUM groups (1KB) share the fp16 groups' pool slot.
            xwT = tsp.tile([P, 8 * P], fp16, tag="TxW", name=f"T_TxW_{i}")
            for g in range(4):
                ps = tp_ps.tile([P, 2 * P], f32, tag="tp",
                                name=f"tpw_{g}_{i}")
                for c in range(2):
                    nc.tensor.transpose(
                        ps[:, c * P:(c + 1) * P],
                        xw_t[:, (2 * g + c) * P:(2 * g + c + 1) * P], ident32)
                nc.scalar.copy(xwT[:, g * 2 * P:(g + 1) * 2 * P], ps)
            stT[i] = (hsT, hdT, xhT, xwT)

        def unit(tag, i):
            return mm_ps.tile([P, HALF], f32, tag="mm", name=f"ps_{tag}_{i}")

        def stage1m(i):
            """A+D matmuls and A-phase elementwise."""
            (hl_t, hr_t, xh_t, xw_t, hl_b, hr_b, xh_b, sS) = st0[i]
            (hsT, hdT, xhT, xwT) = stT[i]

            qS = tinyp.tile([P, 8], f32, tag="qS", name=f"qS_{i}")
            # ql/qr on scalar (Square + accum), cr2 on vector
            sg1 = scr.tile([P, HALF], fp16, tag="scr", name=f"scr_ql_{i}")
            nc.scalar.activation(sg1, hl_t, ACTF.Square,
                                 accum_out=qS[:, 0:1])
            sg2 = scr.tile([P, HALF], fp16, tag="scr", name=f"scr_qr_{i}")
            nc.scalar.activation(sg2, hr_t, ACTF.Square,
                                 accum_out=qS[:, 1:2])
            sg3 = scr.tile([P, HALF], fp16, tag="scr", name=f"scr_cr_{i}")
            nc.vector.scalar_tensor_tensor(sg3, hl_t, 0.0, hr_t, ALU.bypass,
                                           ALU.mult, accum_out=qS[:, 2:3])

            SUq, SBq, TU = unit("SUq", i), unit("SBq", i), unit("TU", i)
            for c in range(4):
                lhs = xhT[:, bass.ts(c, P)]
                st, sp_ = (c == 0), (c == 3)
                nc.tensor.matmul(SUq, lhs, wsb["qWu3"][:, c, :], start=st, stop=sp_)
                nc.tensor.matmul(SBq, lhs, wsb["qWbF"][:, c, :], start=st, stop=sp_)
                nc.tensor.matmul(TU, lhs, wsb["kWu3"][:, c, :], start=st, stop=sp_)
            CD = unit("CD", i)
            for c in range(4):
                nc.tensor.matmul(CD, hdT[:, bass.ts(c, P)],
                                 wsb["kU3"][:, c, :], start=(c == 0),
                                 stop=(c == 3))

            su = att.tile([P, HALF], fp16, tag="su", name=f"su_{i}")
            nc.vector.tensor_add(su, SUq, bc["b_qWu"])
            sbq = att.tile([P, HALF], fp16, tag="sbq", name=f"sbq_{i}")
            nc.vector.tensor_add(sbq, SBq, bc["b_qb"])
            tu = att.tile([P, HALF], fp16, tag="tu", name=f"tu_{i}")
            nc.vector.tensor_add(tu, TU, bc["b_kWu"])
            dk = att.tile([P, HALF], fp16, tag="dk", name=f"dk_{i}")
            nc.vector.tensor_mul(dk, CD, tu)
            u = att.tile([P, HALF], fp16, tag="u", name=f"u_{i}")
            nc.gpsimd.tensor_mul(u, su, dk)
            # uT for the Gv unit (PE transpose + scalar evict)
            uT = tsp.tile([P, 4 * P], fp16, tag="TuT", name=f"TuT_{i}",
                          bufs=2)
            ps_u = tp_ps.tile([P, 4 * P], fp16, tag="tp", name=f"tps_u_{i}")
            for c in range(4):
                nc.tensor.transpose(ps_u[:, c * P:(c + 1) * P],
                                    u[:, c * P:(c + 1) * P], ident)
            nc.scalar.copy(uT, ps_u)

            # c-dot; hl/hr dots come after the Gv unit below
            sdot0 = scr.tile([P, HALF], fp16, tag="scr_b",
                             name=f"scr_dot0_{i}")
            nc.vector.scalar_tensor_tensor(
                sdot0, sbq, 0.0, dk, ALU.bypass, ALU.mult,
                accum_out=qS[:, 3:4])

            # D-phase (attention-independent). Unit order tuned for the
            # 6-slot PSUM rotation: each alloc reuses a bank whose
            # consumer runs early.
            HSU, LSU = unit("HSU", i), unit("LSU", i)
            SBC = unit("SBC", i)
            for c in range(4):
                lhs = xhT[:, bass.ts(c, P)]
                st, sp_ = (c == 0), (c == 3)
                nc.tensor.matmul(HSU, lhs, wsb["hWu3"][:, c, :], start=st, stop=sp_)
                nc.tensor.matmul(LSU, lhs, wsb["lWu3"][:, c, :], start=st, stop=sp_)
                nc.tensor.matmul(SBC, lhs, wsb["WC3"][:, c, :], start=st, stop=sp_)
            # drain the D hyper units early (frees banks for the rotation)
            su_h = phd.tile([P, HALF], fp16, tag="su_h", name=f"su_h_{i}")
            nc.vector.tensor_add(su_h, HSU, bc["b_hWu"])
            su_l = phd.tile([P, HALF], fp16, tag="su_l", name=f"su_l_{i}")
            nc.vector.tensor_add(su_l, LSU, bc["b_lWu"])
            sbc = phd.tile([P, HALF], fp16, tag="sbc", name=f"sbc_{i}")
            nc.vector.tensor_add(sbc, SBC, bc["b_cb"])

            LUp = unit("LU", i)
            for c in range(8):
                nc.tensor.matmul(LUp, xwT[:, bass.ts(c, P)],
                                 wsb["lU3"][:, c, :], start=(c == 0),
                                 stop=(c == 7))
            Mb = unit("Mb", i)
            for c in range(4):
                nc.tensor.matmul(Mb, hsT[:, bass.ts(c, P)],
                                 wsb["WS"][:, c, :], start=(c == 0),
                                 stop=(c == 3))
            D1 = unit("D1", i)
            for c in range(4):
                nc.tensor.matmul(D1, hdT[:, bass.ts(c, P)],
                                 wsb["WT"][:, c, :], start=(c == 0),
                                 stop=(c == 3))
            D2 = unit("D2", i)
            for c in range(4):
                nc.tensor.matmul(D2, hdT[:, bass.ts(c, P)],
                                 wsb["WB"][:, c, :], start=(c == 0),
                                 stop=(c == 3))
            w1 = phd.tile([P, HALF], fp16, tag="w1", name=f"w1_{i}")
            nc.vector.tensor_mul(w1, LUp, su_l)

            # Gv = u @ qU.T; then d0 = c + hl.Gv, d1 = c + hr.Gv
            Gv = unit("Gv", i)
            for c in range(4):
                nc.tensor.matmul(Gv, uT[:, bass.ts(c, P)], wsb["qUT3"][:, c, :],
                                 start=(c == 0), stop=(c == 3))
            for j, hh in enumerate([hl_t, hr_t]):
                sdot = scr.tile([P, HALF], fp16, tag="scr_b",
                                name=f"scr_dot{j + 1}_{i}")
                nc.vector.scalar_tensor_tensor(
                    sdot, Gv, 0.0, hh, ALU.bypass, ALU.mult,
                    accum_out=qS[:, 4 + j:5 + j])
            return (qS, sS, su_h, su_l, sbc, w1, Mb, D1, D2)

        def stage1b(i, h):
            (qS, sS, su_h, su_l, sbc, w1, Mb, D1, D2) = h
            ql, qr, cr2 = qS[:, 0:1], qS[:, 1:2], qS[:, 2:3]
            cdt = qS[:, 3:4]
            sl, sr = sS[:, 0:1], sS[:, 1:2]

            # Mb eviction on scalar (frees its bank early; also needed
            # because DVE reads at most one PSUM operand per op)
            mb_sb = phd.tile([P, HALF], f32, tag="mb_sb", name=f"mb_{i}")
            nc.scalar.copy(mb_sb, Mb)

            # d = [d0, d1] = [c + hl.Gv, c + hr.Gv]; two-term softsign
            # sigmoid -> ab = [p00, -p11]
            dd = tinyp.tile([P, 2], f32, tag="dd", name=f"dd_{i}")
            nc.vector.scalar_tensor_tensor(dd, qS[:, 4:6], 0.0,
                                           cdt.broadcast_to([P, 2]),
                                           ALU.bypass, ALU.add)
            z2 = tinyp.tile([P, 2], f32, tag="z2", name=f"z2_{i}")
            nc.scalar.activation(z2, dd, ACTF.Square, scale=IS)
            sq1 = tinyp.tile([P, 2], f32, tag="sq1", name=f"sq1_{i}")
            nc.scalar.activation(sq1, z2, ACTF.Sqrt, scale=1.0 / SIG_C1,
                                 bias=1.0)
            sq2 = tinyp.tile([P, 2], f32, tag="sq2", name=f"sq2_{i}")
            nc.scalar.activation(sq2, z2, ACTF.Sqrt, scale=1.0 / SIG_C2,
                                 bias=1.0)
            rr = tinyp.tile([P, 4], f32, tag="rr", name=f"rr_{i}")
            nc.vector.reciprocal(rr[:, 0:2], sq1)
            nc.vector.reciprocal(rr[:, 2:4], sq2)
            mm_ = tinyp.tile([P, 2], f32, tag="mm2", name=f"mm2_{i}")
            nc.vector.scalar_tensor_tensor(mm_, rr[:, 0:2], SIG_K1 / SIG_K2,
                                           rr[:, 2:4], ALU.mult, ALU.add)
            psh = tinyp.tile([P, 2], f32, tag="psh", name=f"psh_{i}")
            nc.vector.scalar_tensor_tensor(psh, dd, SIG_K2, mm_,
                                           ALU.mult, ALU.mult)
            ab = tinyp.tile([P, 2], f32, tag="ab", name=f"ab_{i}")
            nc.vector.tensor_add(ab, psh, cb2)
            al, be = ab[:, 0:1], ab[:, 1:2]

            # D1/D2 drained by scalar as soon as al/be exist
            th0 = phd.tile([P, HALF], fp16, tag="th0", name=f"th0_{i}")
            nc.scalar.activation(th0, D1, ACTF.Copy, scale=al)
            th1 = phd.tile([P, HALF], fp16, tag="th1", name=f"th1_{i}")
            nc.scalar.activation(th1, D2, ACTF.Copy, scale=be)
            hv = phd.tile([P, HALF], fp16, tag="hv", name=f"hv_{i}")
            nc.gpsimd.tensor_add(hv, th0, th1)

            # layernorm stats
            gt = tinyp.tile([P, 8], f32, tag="gt", name=f"gt_{i}")
            g_, gh, dl, base = gt[:, 0:1], gt[:, 1:2], gt[:, 2:3], gt[:, 3:4]
            cA, cB, cC = gt[:, 4:5], gt[:, 5:6], gt[:, 6:7]
            nc.vector.tensor_add(g_, al, be)
            nc.vector.tensor_scalar(gh, g_, 0.5, None, ALU.mult)
            sqab = tinyp.tile([P, 2], f32, tag="sqab", name=f"sqab_{i}")
            nc.vector.tensor_mul(sqab, ab, ab)
            nc.vector.tensor_add(dl, sqab[:, 0:1], sqab[:, 1:2])
            nc.vector.tensor_scalar(base, dl, 0.5, 1.0, ALU.mult, ALU.add)
            nc.vector.tensor_add(cA, base, g_)
            nc.vector.scalar_tensor_tensor(cB, g_, -1.0, base, ALU.mult,
                                           ALU.add)
            nc.vector.tensor_scalar(cC, dl, -1.0, 2.0, ALU.mult, ALU.add)
            acc = tinyp.tile([P, 8], f32, tag="acc", name=f"acc_{i}")
            z0, z1, ssqh = acc[:, 0:1], acc[:, 1:2], acc[:, 2:3]
            sh2, sd2, sumxh = acc[:, 3:4], acc[:, 4:5], acc[:, 5:6]
            m2, varh = acc[:, 6:7], acc[:, 7:8]
            nc.vector.tensor_scalar_mul(z0, cr2, cC)
            nc.vector.scalar_tensor_tensor(z1, ql, cA, z0, ALU.mult, ALU.add)
            nc.vector.scalar_tensor_tensor(ssqh, qr, cB, z1, ALU.mult,
                                           ALU.add)
            nc.vector.tensor_add(sh2, sl, sr)
            nc.vector.tensor_sub(sd2, sl, sr)
            nc.vector.scalar_tensor_tensor(sumxh, sd2, gh, sh2, ALU.mult,
                                           ALU.add)
            nc.vector.tensor_mul(m2, sumxh, sumxh)
            nc.vector.scalar_tensor_tensor(varh, m2, -1.0 / 512.0, ssqh,
                                           ALU.mult, ALU.add)
            so = tinyp.tile([P, 2], f32, tag="so", name=f"so_{i}")
            sqstd, rinv = so[:, 0:1], so[:, 1:2]
            nc.scalar.activation(sqstd, varh, ACTF.Sqrt,
                                 scale=2.0 / (DIM - 1))
            nc.vector.reciprocal(rinv, sqstd)

            # t5 = (Mb + hv) - cs*mean ; u1 = rinv * t5
            t5a = phd.tile([P, HALF], f32, tag="t5a", name=f"t5a_{i}")
            nc.vector.scalar_tensor_tensor(t5a, bc["b_cs512"], sumxh, mb_sb,
                                           ALU.mult, ALU.subtract)
            t5 = phd.tile([P, HALF], fp16, tag="t5", name=f"t5_{i}")
            nc.vector.tensor_sub(t5, hv, t5a)
            u1 = phd.tile([P, HALF], fp16, tag="u1", name=f"u1_{i}")
            nc.scalar.activation(u1, t5, ACTF.Copy, scale=rinv)

            v1 = phd.tile([P, HALF], fp16, tag="v1", name=f"v1_{i}")
            nc.gpsimd.tensor_mul(v1, u1, su_h)
            s2 = phd.tile([P, HALF], fp16, tag="s2", name=f"s2_{i}")
            nc.gpsimd.tensor_add(s2, v1, sbc)
            out_t = outp.tile([P, HALF], f32, tag="out_t", name=f"out_{i}")
            nc.gpsimd.tensor_add(out_t, s2, w1)
            nc.sync.dma_start(out_d[bass.ts(i, P), :], out_t)

        stage0(0)
        stage0(1)
        load_weights()
        stageT(0)
        stageT(1)
        for i in range(n_tiles):
            h = stage1m(i)
            if i + 2 < n_tiles:
                stage0(i + 2)
                stageT(i + 2)
            stage1b(i, h)

    nc.compile()
    return nc


_NC_CACHE = {}


def _get_nc(b_loc, mm_dt=None):
    if b_loc not in _NC_CACHE:
        _NC_CACHE[b_loc] = build_nc(b_loc)
    return _NC_CACHE[b_loc]


def make_in_maps(inputs):
    b = inputs["hl"].shape[0]
    b_loc = b // N_CORES
    prep = host_prep(inputs)
    in_maps = []
    for i in range(N_CORES):
        m = {}
        for k in ("hl", "hr", "xw", "xh"):
            v = np.ascontiguousarray(np.asarray(inputs[k], dtype=np.float32))
            m[k] = v[i * b_loc:(i + 1) * b_loc]
        m.update(prep)
        in_maps.append(m)
    return in_maps


def kernel(**inputs):
    b = inputs["hl"].shape[0]
    nc = _get_nc(b // N_CORES)
    in_maps = make_in_maps(inputs)
    res = run_bass_kernel_spmd(nc, in_maps, core_ids=list(range(N_CORES)))
    return np.concatenate([r["out"] for r in res.results], axis=0)
